# revision 14
# baseline (speedup 1.0000x reference)
"""Depthwise 3x3 conv over each depth slice of x[B,H,W,D,C] on 8 trn2 cores.

Strategy (v4 — host-prepped fp16 slab, zero on-chip transposes):
  - Data-parallel over batch: core i handles x[i] ([H,W,D,C] = [64,64,32,64]).
  - The HOST pre-transposes each core's input into 16 depth-pair groups of
    padded fp16 slabs: xs[g][(dp,c)][66 + h*65 + w] with zeroed guard rows
    and pad column, so the device kernel needs no PE transposes, no slab
    copies, and no memsets — every tap is a flat shifted read.
  - fp16 throughout: DMA halves vs f32; DVE tensor_scalar runs in 4x mode
    and tensor_tensor in 2x mode on 2-byte dtypes (fp16 keeps ~2^-11
    relative precision, ~1e-3 total vs the 2e-2 gate).
  - The 64 spatial rows of each group are split across engines
    (40/21/3 steady state; the last two groups lean on PE for the drain):
      rows [0, PE):        9 diag-matmuls (fp16, 1 cyc/row) accumulating in
                           PSUM (chunks of 8 rows = 512-col moving max);
                           ACT copies psum->y2 fp16, folding bias.
      rows [PE, PE+DVE):   head (w*x+b) on ACT; 8 taps as TS(4x mult) +
                           TT(2x add) pairs on DVE, accumulating into y2.
      rows [.., 64):       same TS+TT structure on Pool (gpsimd).
  - Per-group diag matrices (9 x [128,128] fp16) are built by 9 ACT
    activation-scale ops on a fp16 identity (emitted late each iteration
    so they don't delay the psum->y2 copies that recycle PE's banks).
  - Output y2 [128, 4096] fp16 DMAs straight to HBM per band; the host
    inverse-permutes to [B,H,W,D,C] f32.
  - Software pipeline: DMA(p)+diag(p) issued one iteration ahead of the
    conv (p-1); per-band out-DMAs trail within the same iteration.
  - Sem-wait caps handled by the _split_waits post-pass (hoists excess
    waits onto same-engine Drains).
"""

import numpy as np

from contextlib import ExitStack

import concourse.bass as bass
import concourse.mybir as mybir
import concourse.tile as tile
from concourse.masks import make_identity

F32 = mybir.dt.float32
FP16 = mybir.dt.float16

B, H, W, D, C = 8, 64, 64, 32, 64
G = D // 2              # 16 depth-pair groups per core
RS = W + 1              # 65: padded row stride (col 64 of each row is zero)
DATA0 = RS + 1          # 66: flat offset of (h=0, w=0) in the slab
SLAB = DATA0 + 64 * RS + RS + 1   # 66 + 4160 + 66 = 4292

MULT = mybir.AluOpType.mult
ADD = mybir.AluOpType.add
IDENT_F = mybir.ActivationFunctionType.Identity

# taps in (dh, dw) order; index t = (dh+1)*3 + (dw+1)
ALL_TAPS = [(dh, dw) for dh in (-1, 0, 1) for dw in (-1, 0, 1)]

# ---- tunables -----------------------------------------------------------
# rows per group: (pe, dve, h1, h2, pool, e); h1 = ACT products + DVE adds,
# h2 = ACT products + Pool adds, e = ACT products + chained SWDGE accum-DMA
# straight to HBM (adds happen on the DMA engines). Must sum to 64.
PE_ROWS = 40
DVE_ROWS = 21
H1_ROWS = 0
H2_ROWS = 0
E_ROWS = 0
POOL_ROWS = 64 - PE_ROWS - DVE_ROWS - H1_ROWS - H2_ROWS - E_ROWS
E_GROUPS = 11           # groups [0, E_GROUPS) get an E band (chain must
                        # drain ~9 iterations after the group's conv)
XA_BUFS = 3
Y2_BUFS = 3
SC_BUFS = 3
PCV_BUFS = 3
PAIR_CHUNKS = 2         # psum chunks (banks) per pcv tile / ACT copy
DIAG_BUFS = 3
HEADS_ON_ACT = True     # w0*x+b head for DVE/Pool bands on ACT
DIAG_ON = "act"         # engine that scales the identity into diags
PE_BAND_STREAM = True   # per-pair out-DMAs for the PE band
PE_STREAM_TAIL = 2      # apply streaming to the last N groups only
# last two groups lean on PE so the DVE/Pool chains aren't the drain
ROWS_SCHED = [(40, 21, 0, 0, 3, 0)] * 14 + [
    (42, 19, 0, 0, 3, 0),
    (44, 17, 0, 0, 3, 0),
]
WARMUP_MM = 0           # dummy matmuls at t=0 to ramp the PE clock


PE_ROWS_E = 39          # PE/DVE splits for groups that carry an E band
DVE_ROWS_E = 18


def _rows_for(g):
    if ROWS_SCHED is not None:
        return ROWS_SCHED[g]
    if E_ROWS and g < E_GROUPS:
        pe, dve, e = PE_ROWS_E, DVE_ROWS_E, E_ROWS
    else:
        pe, dve, e = PE_ROWS, DVE_ROWS, 0
    pool = 64 - pe - dve - H1_ROWS - H2_ROWS - e
    return (pe, dve, H1_ROWS, H2_ROWS, pool, e)


def _max_rows():
    return tuple(max(_rows_for(g)[i] for g in range(G)) for i in range(6))


def _pe_chunks(pe_rows):
    """Split pe_rows into psum-bank chunks (<=8 rows = 512 f32)."""
    out, r = [], 0
    while r < pe_rows:
        n = min(8, pe_rows - r)
        out.append((r, n))
        r += n
    return out


def _build_nc():
    nc = bass.Bass("TRN2", target_bir_lowering=False, debug=False)
    xs = nc.dram_tensor("xs", [G, 128, SLAB], FP16, kind="ExternalInput").ap()
    wbs = nc.dram_tensor("wbs", [128, G * 9 + G], F32, kind="ExternalInput").ap()
    ys = nc.dram_tensor("ys", [G, 128, 4096], FP16, kind="ExternalOutput").ap()

    with tile.TileContext(nc) as tc, ExitStack() as ctx:
        consts = ctx.enter_context(tc.tile_pool(name="consts", bufs=1))
        identf = consts.tile([128, 128], F32, name="identf")
        make_identity(nc, identf[:])
        ident16 = consts.tile([128, 128], FP16, name="ident16")
        nc.scalar.copy(ident16[:], identf[:])
        wbst = consts.tile([128, G * 9 + G], F32, name="wbst")
        # group-0 tap weights first: unblocks diag(0) while the bulk loads
        nc.sync.dma_start(wbst[:, 0:9], wbs[:, 0:9])
        wst = wbst[:, 0 : G * 9]
        bst = wbst[:, G * 9 : G * 9 + G]

        xap = ctx.enter_context(tc.tile_pool(name="xa", bufs=XA_BUFS))
        y2p = ctx.enter_context(tc.tile_pool(name="y2", bufs=Y2_BUFS))
        scp = ctx.enter_context(tc.tile_pool(name="scr", bufs=SC_BUFS))
        dgp = ctx.enter_context(tc.tile_pool(name="diag", bufs=DIAG_BUFS))
        pcv = ctx.enter_context(
            tc.tile_pool(name="pcv", bufs=PCV_BUFS, space=bass.MemorySpace.PSUM)
        )

        if WARMUP_MM:
            pwm = ctx.enter_context(
                tc.tile_pool(name="pwm", bufs=1, space=bass.MemorySpace.PSUM)
            )
            warm_in = consts.tile([128, 512], FP16, name="warm_in")
            nc.gpsimd.memset(warm_in[:], 0.0)
            wq = pwm.tile([128, 512], F32, name="wq")
            for _ in range(WARMUP_MM):
                nc.tensor.matmul(wq[:], ident16[:], warm_in[:],
                                 start=True, stop=True)

        in_state = {}
        conv_state = {}
        e_state = {}
        sep = None
        if _max_rows()[5]:
            # per-tap slot pools: slot t is read by its chain link t
            # iterations after being written -> deeper pools for later taps
            sep = [
                ctx.enter_context(tc.tile_pool(name=f"se{t}", bufs=t + 3))
                for t in range(9)
            ]

        def wap(g, t):
            i = g * 9 + t
            return wst[:, i : i + 1]

        def dma_issue(g):
            xa = xap.tile([128, SLAB], FP16, tag="xa")
            if g == 0:
                # split so the PE band's first chunks unblock early
                head = DATA0 + 10 * RS + RS + 1
                nc.sync.dma_start(xa[:, 0:head], xs[g][:, 0:head])
                nc.sync.dma_start(xa[:, head:SLAB], xs[g][:, head:SLAB])
                nc.sync.dma_start(wbst[:, 9:], wbs[:, 9:])
            else:
                nc.sync.dma_start(xa[:], xs[g])
            in_state[g] = dict(xa=xa)

        def diag_build(g, eng=None):
            eng = eng or DIAG_ON
            diag = dgp.tile([128, 9 * 128], FP16, tag="diag")
            for t in range(9):
                if eng == "act":
                    nc.scalar.activation(
                        diag[:, t * 128 : (t + 1) * 128], ident16[:],
                        IDENT_F, scale=wap(g, t),
                    )
                else:
                    nc.vector.tensor_scalar(
                        diag[:, t * 128 : (t + 1) * 128], ident16[:], wap(g, t),
                        None, MULT,
                    )
            in_state[g]["diag"] = diag

        def xsh(g, dh, dw, r0, nr):
            xa = in_state[g]["xa"]
            s0 = DATA0 + dh * RS + dw + r0 * RS
            v = xa[:, s0 : s0 + nr * RS]
            return v.rearrange("p (r b) -> p r b", b=RS)[:, :, 0:64]

        def pe_conv(g):
            st = in_state[g]
            diag = st["diag"]
            bias = bst[:, g : g + 1]
            y2 = y2p.tile([128, 4096], FP16, tag="y2")
            conv_state[g] = dict(y2=y2)
            pe_r = _rows_for(g)[0]
            chunks = _pe_chunks(pe_r)
            # group chunks into PAIR_CHUNKS-bank psum tiles: one ACT copy each
            stream = PE_BAND_STREAM and g >= G - PE_STREAM_TAIL
            k = 0
            while k < len(chunks):
                take = PAIR_CHUNKS
                if stream and any(n != 8 for _, n in chunks[k : k + take]):
                    take = 1
                pair = chunks[k : k + take]
                cols = sum(nr for _, nr in pair) * 64
                Pq = pcv.tile([128, 512 * PAIR_CHUNKS], F32, tag="pcv")
                off = 0
                for r0, nr in pair:
                    for t, (dh, dw) in enumerate(ALL_TAPS):
                        nc.tensor.matmul(
                            Pq[:, off : off + nr * 64],
                            diag[:, 128 * t : 128 * (t + 1)],
                            xsh(g, dh, dw, r0, nr),
                            start=(t == 0), stop=(t == 8),
                        )
                    off += nr * 64
                k += take
                r0_first = pair[0][0]
                nc.scalar.activation(
                    y2[:, r0_first * 64 : r0_first * 64 + cols].rearrange(
                        "p (r w) -> p r w", w=64
                    ),
                    Pq[:, 0:cols].rearrange("p (r w) -> p r w", w=64),
                    IDENT_F, bias=bias,
                )
                if stream:
                    # ship this pair's rows as soon as the copy lands, so
                    # the group's final out-DMA is only the last sub-band
                    nc.sync.dma_start(
                        ys[g][:, r0_first * 64 : r0_first * 64 + cols],
                        y2[:, r0_first * 64 : r0_first * 64 + cols],
                    )

        def chains(g):
            st = in_state[g]
            bias = bst[:, g : g + 1]
            y2 = conv_state[g]["y2"]
            pe_r, dve_r, h1_r, h2_r, pool_r, e_r = _rows_for(g)
            r_dve = pe_r
            r_h = pe_r + dve_r
            r_pool = r_h + h1_r + h2_r

            # E band: 9 products on ACT into per-tap slots; a chained
            # accum-DMA (one link per later iteration) sums them in HBM
            if e_r:
                r_e = 64 - e_r
                slots = []
                for t, (dh, dw) in enumerate(ALL_TAPS):
                    se = sep[t].tile([128, e_r * 64], FP16, tag=f"se{t}")
                    nc.scalar.activation(
                        se[:].rearrange("p (r w) -> p r w", w=64),
                        xsh(g, dh, dw, r_e, e_r),
                        IDENT_F, bias=(bias if t == 0 else 0.0),
                        scale=wap(g, t),
                    )
                    slots.append(se)
                e_state[g] = dict(slots=slots, last=None)

            def yv(r0, nr):
                return y2[:, r0 * 64 : (r0 + nr) * 64].rearrange(
                    "p (r w) -> p r w", w=64
                )

            (h0, w0), rest = ALL_TAPS[0], ALL_TAPS[1:]

            # H band: 9 products on ACT into scratch slots, adds on DVE (h1)
            # and Pool (h2). Products are emitted first (ACT stream); the
            # adds are emitted after the A/Pool chains so the cross-engine
            # waits are long satisfied by then.
            h_r = h1_r + h2_r
            slot = None
            if h_r:
                mh = _max_rows()[2] + _max_rows()[3]
                sch = scp.tile([128, 9 * mh * 64], FP16, tag="sch")
                slot = [sch[:, t * mh * 64 : t * mh * 64 + h_r * 64]
                        for t in range(9)]
                for t, (dh, dw) in enumerate(ALL_TAPS):
                    nc.scalar.activation(
                        slot[t].rearrange("p (r w) -> p r w", w=64),
                        xsh(g, dh, dw, r_h, h_r),
                        IDENT_F, bias=(bias if t == 0 else 0.0),
                        scale=wap(g, t),
                    )

            if dve_r:
                if HEADS_ON_ACT:
                    nc.scalar.activation(
                        yv(r_dve, dve_r), xsh(g, h0, w0, r_dve, dve_r),
                        IDENT_F, bias=bias, scale=wap(g, 0),
                    )
                else:
                    nc.vector.tensor_scalar(
                        yv(r_dve, dve_r), xsh(g, h0, w0, r_dve, dve_r),
                        wap(g, 0), bias, MULT, ADD,
                    )
                scr = scp.tile([128, _max_rows()[1] * 64], FP16, tag="scr")
                sv = scr[:, 0 : dve_r * 64].rearrange("p (r w) -> p r w", w=64)
                acc = yv(r_dve, dve_r)
                for t, (dh, dw) in enumerate(rest, start=1):
                    nc.vector.tensor_scalar(
                        sv, xsh(g, dh, dw, r_dve, dve_r), wap(g, t), None, MULT,
                    )
                    nc.vector.tensor_tensor(acc, acc, sv, ADD)

            if pool_r:
                if HEADS_ON_ACT:
                    nc.scalar.activation(
                        yv(r_pool, pool_r), xsh(g, h0, w0, r_pool, pool_r),
                        IDENT_F, bias=bias, scale=wap(g, 0),
                    )
                else:
                    nc.gpsimd.tensor_scalar(
                        yv(r_pool, pool_r), xsh(g, h0, w0, r_pool, pool_r),
                        wap(g, 0), bias, MULT, ADD,
                    )
                scq = scp.tile([128, (_max_rows()[4] + 2) * 64], FP16, tag="scq")
                qv = scq[:, 0 : pool_r * 64].rearrange("p (r w) -> p r w", w=64)
                accp = yv(r_pool, pool_r)
                for t, (dh, dw) in enumerate(rest, start=1):
                    nc.gpsimd.tensor_scalar(
                        qv, xsh(g, dh, dw, r_pool, pool_r), wap(g, t), None, MULT,
                    )
                    nc.gpsimd.tensor_tensor(accp, accp, qv, ADD)

            n1 = h1_r * 64
            if h1_r:
                acc1 = y2[:, r_h * 64 : r_h * 64 + n1]
                nc.vector.tensor_tensor(
                    acc1, slot[0][:, 0:n1], slot[1][:, 0:n1], ADD)
                for t in range(2, 9):
                    nc.vector.tensor_tensor(acc1, acc1, slot[t][:, 0:n1], ADD)
            if h2_r:
                acc2 = y2[:, r_h * 64 + n1 : (r_h + h_r) * 64]
                nc.gpsimd.tensor_tensor(
                    acc2, slot[0][:, n1:], slot[1][:, n1:], ADD)
                for t in range(2, 9):
                    nc.gpsimd.tensor_tensor(acc2, acc2, slot[t][:, n1:], ADD)

        def out_path(g):
            st = conv_state.pop(g)
            y2 = st["y2"]
            pe_r, dve_r, h1_r, h2_r, pool_r, e_r = _rows_for(g)
            bands = []
            if pe_r and not (PE_BAND_STREAM and g >= G - PE_STREAM_TAIL):
                bands.append((0, pe_r))
            if dve_r + h1_r:
                bands.append((pe_r, dve_r + h1_r))
            if h2_r + pool_r:
                bands.append((pe_r + dve_r + h1_r, h2_r + pool_r))
            for r0, nr in bands:
                nc.sync.dma_start(
                    ys[g][:, r0 * 64 : (r0 + nr) * 64],
                    y2[:, r0 * 64 : (r0 + nr) * 64],
                )

        def e_link(g, t):
            st = e_state[g]
            e_r = _rows_for(g)[5]
            r_e = 64 - e_r
            dst = ys[g][:, r_e * 64 : 4096]
            src = st["slots"][t]
            if t == 0:
                d = nc.gpsimd.dma_start(dst, src[:])
            else:
                d = nc.gpsimd.dma_start(dst, src[:], accum_op=ADD)
                add_dep_helper(d.ins, st["last"].ins, reason="accum-chain")
            st["last"] = d
            if t == 8:
                e_state.pop(g)

        flush = 8 if (E_ROWS or ROWS_SCHED) else 0
        for p in range(G + 1 + flush):
            if p < G:
                dma_issue(p)
                if DIAG_ON != "act" or p == 0:
                    diag_build(p, eng="dve" if p == 0 else None)
            if 1 <= p <= G:
                g = p - 1
                pe_conv(g)
                chains(g)
                out_path(g)
            if DIAG_ON == "act" and 0 < p < G:
                diag_build(p)
            for gl in sorted(e_state.keys()):
                t = p - 1 - gl
                if 0 <= t <= 8:
                    e_link(gl, t)

    return nc


# walrus setupSyncWait caps per engine struct: hoist excess waits onto
# injected same-engine Drains (Tile's epilogue Drain carries many waits,
# so Drain accepts them).
_WAIT_CAPS = {"PE": 1, "Activation": 1, "DVE": 1, "Pool": 1, "SP": 1}
_SPLIT_SEQ = [0]


def _split_waits(nc):
    fn = nc.m.functions[0]
    nsplit = 0
    for blk in fn.blocks:
        out = []
        changed = False
        for ins in blk.instructions:
            si = ins.sync_info
            waits = list(si.on_wait) if si is not None and si.on_wait else []
            eng = getattr(ins, "engine", None)
            engname = getattr(eng, "value", None) or str(eng)
            cap = _WAIT_CAPS.get(engname)
            if cap is not None and len(waits) > cap:
                excess, keep = waits[:-cap], waits[-cap:]
                for w in excess:
                    _SPLIT_SEQ[0] += 1
                    d = mybir.InstDrain(name=f"I-ws{_SPLIT_SEQ[0]}", ins=[], outs=[])
                    d.engine = eng
                    d.sync_info = mybir.SyncInfo(on_wait=[w], on_update=[])
                    out.append(d)
                ins.sync_info = mybir.SyncInfo(
                    on_wait=keep, on_update=list(si.on_update or [])
                )
                changed = True
                nsplit += 1
            out.append(ins)
        if changed:
            blk.instructions = out
    return nsplit


_NC_CACHE = None


def _get_nc():
    global _NC_CACHE
    if _NC_CACHE is None:
        nc = _build_nc()
        _split_waits(nc)
        _NC_CACHE = nc
    return _NC_CACHE


class Runner:
    """Persistent PJRT executor for an SPMD bass module (axon path)."""

    def __init__(self, nc, n_cores=8):
        import jax
        from jax.experimental.shard_map import shard_map
        from jax.sharding import Mesh, PartitionSpec
        from concourse import bass2jax

        bass2jax.install_neuronx_cc_hook()
        self.jax = jax
        self.nc = nc
        self.n = n_cores
        partition_name = (
            nc.partition_id_tensor.name if nc.partition_id_tensor else None
        )
        in_names, out_names, out_avals = [], [], []
        for alloc in nc.m.functions[0].allocations:
            if not isinstance(alloc, mybir.MemoryLocationSet):
                continue
            name = alloc.memorylocations[0].name
            if alloc.kind == "ExternalInput":
                if name != partition_name:
                    in_names.append(name)
            elif alloc.kind == "ExternalOutput":
                out_names.append(name)
                out_avals.append(
                    jax.core.ShapedArray(
                        tuple(alloc.tensor_shape), mybir.dt.np(alloc.dtype)
                    )
                )
        self.in_names = list(in_names)
        self.out_names = out_names
        self.out_avals = out_avals
        bind_in_names = list(in_names) + list(out_names)
        if partition_name is not None:
            bind_in_names.append(partition_name)
        bind_in_names = tuple(bind_in_names)
        n_params = len(in_names)
        n_outs = len(out_names)

        def _body(*args):
            operands = list(args)
            if partition_name is not None:
                operands.append(bass2jax.partition_id_tensor())
            outs = bass2jax._bass_exec_p.bind(
                *operands,
                out_avals=tuple(out_avals),
                in_names=bind_in_names,
                out_names=tuple(out_names),
                lowering_input_output_aliases=(),
                sim_require_finite=True,
                sim_require_nnan=True,
                nc=nc,
            )
            return tuple(outs)

        devices = jax.devices()[:n_cores]
        self.mesh = Mesh(np.asarray(devices), ("core",))
        self.spec = PartitionSpec("core")
        in_specs = (self.spec,) * (n_params + n_outs)
        out_specs = (self.spec,) * n_outs
        donate = tuple(range(n_params, n_params + n_outs))
        self.fn = jax.jit(
            shard_map(
                _body,
                mesh=self.mesh,
                in_specs=in_specs,
                out_specs=out_specs,
                check_rep=False,
            ),
            donate_argnums=donate,
            keep_unused=True,
        )
        sharding = jax.sharding.NamedSharding(self.mesh, self.spec)
        self.zeros_fn = jax.jit(
            lambda: tuple(
                self.jax.numpy.zeros((n_cores * a.shape[0], *a.shape[1:]), a.dtype)
                for a in out_avals
            ),
            out_shardings=(sharding,) * n_outs,
        )

    def put_inputs(self, in_maps):
        jax = self.jax
        sharding = jax.sharding.NamedSharding(self.mesh, self.spec)
        arrs = []
        for name in self.in_names:
            cat = np.concatenate([np.asarray(m[name]) for m in in_maps], axis=0)
            arrs.append(jax.device_put(cat, sharding))
        jax.block_until_ready(arrs)
        return arrs

    def __call__(self, dev_inputs):
        zs = self.zeros_fn()
        self.jax.block_until_ready(zs)
        out = self.fn(*dev_inputs, *zs)
        self.jax.block_until_ready(out)
        return out

    def time_it(self, dev_inputs, reps=10):
        import time as _t

        ts = []
        for _ in range(reps):
            zs = self.zeros_fn()
            self.jax.block_until_ready(zs)
            t0 = _t.perf_counter()
            out = self.fn(*dev_inputs, *zs)
            self.jax.block_until_ready(out)
            ts.append(_t.perf_counter() - t0)
        return ts

    def to_numpy(self, out):
        n = self.n
        return [
            {
                name: np.asarray(out[i]).reshape(n, *self.out_avals[i].shape)[c]
                for i, name in enumerate(self.out_names)
            }
            for c in range(n)
        ]


_RUNNER = None


def _get_runner():
    global _RUNNER
    if _RUNNER is None:
        _RUNNER = Runner(_get_nc(), B)
    return _RUNNER


def _prep_wb(w, b):
    # ws[p, g*9 + t] = w[2g + p//64, t//3, t%3, p%64]
    w = np.asarray(w, dtype=np.float32).reshape(G, 2, 9, C)
    ws = np.ascontiguousarray(w.transpose(1, 3, 0, 2).reshape(128, G * 9))
    b = np.asarray(b, dtype=np.float32).reshape(G, 2, C)
    bs = np.ascontiguousarray(b.transpose(1, 2, 0).reshape(128, G))
    return ws, bs


def _prep_x(xi):
    """[H,W,D,C] f32 -> [G, 128, SLAB] fp16 padded slab."""
    # (h, w, d, c) -> (g, dp, c, h, w)
    xt = xi.transpose(2, 3, 0, 1).reshape(G, 2, C, H, W)
    slab = np.zeros((G, 2, C, SLAB), dtype=np.float16)
    sv = slab[:, :, :, DATA0 : DATA0 + 64 * RS].reshape(G, 2, C, 64, RS)
    sv[:, :, :, :, 0:64] = xt.astype(np.float16)
    return slab.reshape(G, 128, SLAB)


def _post_y(ysg):
    """[G, 128, 4096] fp16 -> [H,W,D,C] f32."""
    y = ysg.reshape(G, 2, C, H, W).astype(np.float32)
    # (g, dp, c, h, w) -> (h, w, g, dp, c) -> [H, W, D, C]
    return np.ascontiguousarray(y.transpose(3, 4, 0, 1, 2).reshape(H, W, D, C))


def _in_maps(inputs):
    x = np.asarray(inputs["x"], dtype=np.float32)
    ws, bs = _prep_wb(inputs["w"], inputs["b"])
    wbs = np.ascontiguousarray(np.concatenate([ws, bs], axis=1))
    return [{"xs": _prep_x(x[i]), "wbs": wbs} for i in range(B)]


def kernel(**inputs) -> np.ndarray:
    r = _get_runner()
    dev = r.put_inputs(_in_maps(inputs))
    res = r.to_numpy(r(dev))
    return np.stack([_post_y(m["ys"]) for m in res], axis=0)


# revision 15
# speedup vs baseline: 1.0116x; 1.0116x over previous
"""Depthwise 3x3 conv over each depth slice of x[B,H,W,D,C] on 8 trn2 cores.

Strategy (v4 — host-prepped fp16 slab, zero on-chip transposes):
  - Data-parallel over batch: core i handles x[i] ([H,W,D,C] = [64,64,32,64]).
  - The HOST pre-transposes each core's input into 16 depth-pair groups of
    padded fp16 slabs: xs[g][(dp,c)][66 + h*65 + w] with zeroed guard rows
    and pad column, so the device kernel needs no PE transposes, no slab
    copies, and no memsets — every tap is a flat shifted read.
  - fp16 throughout: DMA halves vs f32; DVE tensor_scalar runs in 4x mode
    and tensor_tensor in 2x mode on 2-byte dtypes (fp16 keeps ~2^-11
    relative precision, ~1e-3 total vs the 2e-2 gate).
  - The 64 spatial rows of each group are split across engines
    (40/21/3 steady state; the last two groups lean on PE for the drain):
      rows [0, PE):        9 diag-matmuls (fp16, 1 cyc/row) accumulating in
                           PSUM (chunks of 8 rows = 512-col moving max);
                           ACT copies psum->y2 fp16, folding bias.
      rows [PE, PE+DVE):   head (w*x+b) on ACT; 8 taps as TS(4x mult) +
                           TT(2x add) pairs on DVE, accumulating into y2.
      rows [.., 64):       same TS+TT structure on Pool (gpsimd).
  - Per-group diag matrices (9 x [128,128] fp16) are built by 9 ACT
    activation-scale ops on a fp16 identity (emitted late each iteration
    so they don't delay the psum->y2 copies that recycle PE's banks).
  - Output y2 [128, 4096] fp16 DMAs straight to HBM per band; the host
    inverse-permutes to [B,H,W,D,C] f32.
  - Software pipeline: DMA(p)+diag(p) issued one iteration ahead of the
    conv (p-1); per-band out-DMAs trail within the same iteration.
  - Sem-wait caps handled by the _split_waits post-pass (hoists excess
    waits onto same-engine Drains).
"""

import numpy as np

from contextlib import ExitStack

import concourse.bass as bass
import concourse.mybir as mybir
import concourse.tile as tile
from concourse.masks import make_identity

F32 = mybir.dt.float32
FP16 = mybir.dt.float16

B, H, W, D, C = 8, 64, 64, 32, 64
G = D // 2              # 16 depth-pair groups per core
RS = W + 1              # 65: padded row stride (col 64 of each row is zero)
DATA0 = RS + 1          # 66: flat offset of (h=0, w=0) in the slab
SLAB = DATA0 + 64 * RS + RS + 1   # 66 + 4160 + 66 = 4292

MULT = mybir.AluOpType.mult
ADD = mybir.AluOpType.add
IDENT_F = mybir.ActivationFunctionType.Identity

# taps in (dh, dw) order; index t = (dh+1)*3 + (dw+1)
ALL_TAPS = [(dh, dw) for dh in (-1, 0, 1) for dw in (-1, 0, 1)]

# ---- tunables -----------------------------------------------------------
# rows per group: (pe, dve, h1, h2, pool, e); h1 = ACT products + DVE adds,
# h2 = ACT products + Pool adds, e = ACT products + chained SWDGE accum-DMA
# straight to HBM (adds happen on the DMA engines). Must sum to 64.
PE_ROWS = 40
DVE_ROWS = 21
H1_ROWS = 0
H2_ROWS = 0
E_ROWS = 0
POOL_ROWS = 64 - PE_ROWS - DVE_ROWS - H1_ROWS - H2_ROWS - E_ROWS
E_GROUPS = 11           # groups [0, E_GROUPS) get an E band (chain must
                        # drain ~9 iterations after the group's conv)
XA_BUFS = 3
Y2_BUFS = 3
SC_BUFS = 3
PCV_BUFS = 3
PAIR_CHUNKS = 2         # psum chunks (banks) per pcv tile / ACT copy
DIAG_BUFS = 3
HEADS_ON_ACT = True     # w0*x+b head for DVE/Pool bands on ACT
DIAG_ON = "act"         # engine that scales the identity into diags
PE_BAND_STREAM = True   # per-pair out-DMAs for the PE band
PE_STREAM_TAIL = 2      # apply streaming to the last N groups only
# alternate (40,21,3) with (39,21,4): Pool absorbs a 4th row every other
# group (its serial chain needs the recovery iteration in between), which
# trims total PE rows; last two groups lean on PE so the DVE/Pool chains
# aren't the pipeline drain
ROWS_SCHED = [(40, 21, 0, 0, 3, 0), (39, 21, 0, 0, 4, 0)] * 7 + [
    (41, 19, 0, 0, 4, 0),
    (44, 16, 0, 0, 4, 0),
]
WARMUP_MM = 0           # dummy matmuls at t=0 to ramp the PE clock


PE_ROWS_E = 39          # PE/DVE splits for groups that carry an E band
DVE_ROWS_E = 18


def _rows_for(g):
    if ROWS_SCHED is not None:
        return ROWS_SCHED[g]
    if E_ROWS and g < E_GROUPS:
        pe, dve, e = PE_ROWS_E, DVE_ROWS_E, E_ROWS
    else:
        pe, dve, e = PE_ROWS, DVE_ROWS, 0
    pool = 64 - pe - dve - H1_ROWS - H2_ROWS - e
    return (pe, dve, H1_ROWS, H2_ROWS, pool, e)


def _max_rows():
    return tuple(max(_rows_for(g)[i] for g in range(G)) for i in range(6))


def _pe_chunks(pe_rows):
    """Split pe_rows into psum-bank chunks (<=8 rows = 512 f32)."""
    out, r = [], 0
    while r < pe_rows:
        n = min(8, pe_rows - r)
        out.append((r, n))
        r += n
    return out


def _build_nc():
    nc = bass.Bass("TRN2", target_bir_lowering=False, debug=False)
    xs = nc.dram_tensor("xs", [G, 128, SLAB], FP16, kind="ExternalInput").ap()
    wbs = nc.dram_tensor("wbs", [128, G * 9 + G], F32, kind="ExternalInput").ap()
    ys = nc.dram_tensor("ys", [G, 128, 4096], FP16, kind="ExternalOutput").ap()

    with tile.TileContext(nc) as tc, ExitStack() as ctx:
        consts = ctx.enter_context(tc.tile_pool(name="consts", bufs=1))
        identf = consts.tile([128, 128], F32, name="identf")
        make_identity(nc, identf[:])
        ident16 = consts.tile([128, 128], FP16, name="ident16")
        nc.scalar.copy(ident16[:], identf[:])
        wbst = consts.tile([128, G * 9 + G], F32, name="wbst")
        # group-0 tap weights first: unblocks diag(0) while the bulk loads
        nc.sync.dma_start(wbst[:, 0:9], wbs[:, 0:9])
        wst = wbst[:, 0 : G * 9]
        bst = wbst[:, G * 9 : G * 9 + G]

        xap = ctx.enter_context(tc.tile_pool(name="xa", bufs=XA_BUFS))
        y2p = ctx.enter_context(tc.tile_pool(name="y2", bufs=Y2_BUFS))
        scp = ctx.enter_context(tc.tile_pool(name="scr", bufs=SC_BUFS))
        dgp = ctx.enter_context(tc.tile_pool(name="diag", bufs=DIAG_BUFS))
        pcv = ctx.enter_context(
            tc.tile_pool(name="pcv", bufs=PCV_BUFS, space=bass.MemorySpace.PSUM)
        )

        if WARMUP_MM:
            pwm = ctx.enter_context(
                tc.tile_pool(name="pwm", bufs=1, space=bass.MemorySpace.PSUM)
            )
            warm_in = consts.tile([128, 512], FP16, name="warm_in")
            nc.gpsimd.memset(warm_in[:], 0.0)
            wq = pwm.tile([128, 512], F32, name="wq")
            for _ in range(WARMUP_MM):
                nc.tensor.matmul(wq[:], ident16[:], warm_in[:],
                                 start=True, stop=True)

        in_state = {}
        conv_state = {}
        e_state = {}
        sep = None
        if _max_rows()[5]:
            # per-tap slot pools: slot t is read by its chain link t
            # iterations after being written -> deeper pools for later taps
            sep = [
                ctx.enter_context(tc.tile_pool(name=f"se{t}", bufs=t + 3))
                for t in range(9)
            ]

        def wap(g, t):
            i = g * 9 + t
            return wst[:, i : i + 1]

        def dma_issue(g):
            xa = xap.tile([128, SLAB], FP16, tag="xa")
            if g == 0:
                # split so the PE band's first chunks unblock early
                head = DATA0 + 10 * RS + RS + 1
                nc.sync.dma_start(xa[:, 0:head], xs[g][:, 0:head])
                nc.sync.dma_start(xa[:, head:SLAB], xs[g][:, head:SLAB])
                nc.sync.dma_start(wbst[:, 9:], wbs[:, 9:])
            else:
                nc.sync.dma_start(xa[:], xs[g])
            in_state[g] = dict(xa=xa)

        def diag_build(g, eng=None):
            eng = eng or DIAG_ON
            diag = dgp.tile([128, 9 * 128], FP16, tag="diag")
            for t in range(9):
                if eng == "act":
                    nc.scalar.activation(
                        diag[:, t * 128 : (t + 1) * 128], ident16[:],
                        IDENT_F, scale=wap(g, t),
                    )
                else:
                    nc.vector.tensor_scalar(
                        diag[:, t * 128 : (t + 1) * 128], ident16[:], wap(g, t),
                        None, MULT,
                    )
            in_state[g]["diag"] = diag

        def xsh(g, dh, dw, r0, nr):
            xa = in_state[g]["xa"]
            s0 = DATA0 + dh * RS + dw + r0 * RS
            v = xa[:, s0 : s0 + nr * RS]
            return v.rearrange("p (r b) -> p r b", b=RS)[:, :, 0:64]

        def pe_conv(g):
            st = in_state[g]
            diag = st["diag"]
            bias = bst[:, g : g + 1]
            y2 = y2p.tile([128, 4096], FP16, tag="y2")
            conv_state[g] = dict(y2=y2)
            pe_r = _rows_for(g)[0]
            chunks = _pe_chunks(pe_r)
            # group chunks into PAIR_CHUNKS-bank psum tiles: one ACT copy each
            stream = PE_BAND_STREAM and g >= G - PE_STREAM_TAIL
            k = 0
            while k < len(chunks):
                take = PAIR_CHUNKS
                if stream and any(n != 8 for _, n in chunks[k : k + take]):
                    take = 1
                pair = chunks[k : k + take]
                cols = sum(nr for _, nr in pair) * 64
                Pq = pcv.tile([128, 512 * PAIR_CHUNKS], F32, tag="pcv")
                off = 0
                for r0, nr in pair:
                    for t, (dh, dw) in enumerate(ALL_TAPS):
                        nc.tensor.matmul(
                            Pq[:, off : off + nr * 64],
                            diag[:, 128 * t : 128 * (t + 1)],
                            xsh(g, dh, dw, r0, nr),
                            start=(t == 0), stop=(t == 8),
                        )
                    off += nr * 64
                k += take
                r0_first = pair[0][0]
                nc.scalar.activation(
                    y2[:, r0_first * 64 : r0_first * 64 + cols].rearrange(
                        "p (r w) -> p r w", w=64
                    ),
                    Pq[:, 0:cols].rearrange("p (r w) -> p r w", w=64),
                    IDENT_F, bias=bias,
                )
                if stream:
                    # ship this pair's rows as soon as the copy lands, so
                    # the group's final out-DMA is only the last sub-band
                    nc.sync.dma_start(
                        ys[g][:, r0_first * 64 : r0_first * 64 + cols],
                        y2[:, r0_first * 64 : r0_first * 64 + cols],
                    )

        def chains(g):
            st = in_state[g]
            bias = bst[:, g : g + 1]
            y2 = conv_state[g]["y2"]
            pe_r, dve_r, h1_r, h2_r, pool_r, e_r = _rows_for(g)
            r_dve = pe_r
            r_h = pe_r + dve_r
            r_pool = r_h + h1_r + h2_r

            # E band: 9 products on ACT into per-tap slots; a chained
            # accum-DMA (one link per later iteration) sums them in HBM
            if e_r:
                r_e = 64 - e_r
                slots = []
                for t, (dh, dw) in enumerate(ALL_TAPS):
                    se = sep[t].tile([128, e_r * 64], FP16, tag=f"se{t}")
                    nc.scalar.activation(
                        se[:].rearrange("p (r w) -> p r w", w=64),
                        xsh(g, dh, dw, r_e, e_r),
                        IDENT_F, bias=(bias if t == 0 else 0.0),
                        scale=wap(g, t),
                    )
                    slots.append(se)
                e_state[g] = dict(slots=slots, last=None)

            def yv(r0, nr):
                return y2[:, r0 * 64 : (r0 + nr) * 64].rearrange(
                    "p (r w) -> p r w", w=64
                )

            (h0, w0), rest = ALL_TAPS[0], ALL_TAPS[1:]

            # H band: 9 products on ACT into scratch slots, adds on DVE (h1)
            # and Pool (h2). Products are emitted first (ACT stream); the
            # adds are emitted after the A/Pool chains so the cross-engine
            # waits are long satisfied by then.
            h_r = h1_r + h2_r
            slot = None
            if h_r:
                mh = _max_rows()[2] + _max_rows()[3]
                sch = scp.tile([128, 9 * mh * 64], FP16, tag="sch")
                slot = [sch[:, t * mh * 64 : t * mh * 64 + h_r * 64]
                        for t in range(9)]
                for t, (dh, dw) in enumerate(ALL_TAPS):
                    nc.scalar.activation(
                        slot[t].rearrange("p (r w) -> p r w", w=64),
                        xsh(g, dh, dw, r_h, h_r),
                        IDENT_F, bias=(bias if t == 0 else 0.0),
                        scale=wap(g, t),
                    )

            if dve_r:
                if HEADS_ON_ACT:
                    nc.scalar.activation(
                        yv(r_dve, dve_r), xsh(g, h0, w0, r_dve, dve_r),
                        IDENT_F, bias=bias, scale=wap(g, 0),
                    )
                else:
                    nc.vector.tensor_scalar(
                        yv(r_dve, dve_r), xsh(g, h0, w0, r_dve, dve_r),
                        wap(g, 0), bias, MULT, ADD,
                    )
                scr = scp.tile([128, _max_rows()[1] * 64], FP16, tag="scr")
                sv = scr[:, 0 : dve_r * 64].rearrange("p (r w) -> p r w", w=64)
                acc = yv(r_dve, dve_r)
                for t, (dh, dw) in enumerate(rest, start=1):
                    nc.vector.tensor_scalar(
                        sv, xsh(g, dh, dw, r_dve, dve_r), wap(g, t), None, MULT,
                    )
                    nc.vector.tensor_tensor(acc, acc, sv, ADD)

            if pool_r:
                if HEADS_ON_ACT:
                    nc.scalar.activation(
                        yv(r_pool, pool_r), xsh(g, h0, w0, r_pool, pool_r),
                        IDENT_F, bias=bias, scale=wap(g, 0),
                    )
                else:
                    nc.gpsimd.tensor_scalar(
                        yv(r_pool, pool_r), xsh(g, h0, w0, r_pool, pool_r),
                        wap(g, 0), bias, MULT, ADD,
                    )
                scq = scp.tile([128, (_max_rows()[4] + 2) * 64], FP16, tag="scq")
                qv = scq[:, 0 : pool_r * 64].rearrange("p (r w) -> p r w", w=64)
                accp = yv(r_pool, pool_r)
                for t, (dh, dw) in enumerate(rest, start=1):
                    nc.gpsimd.tensor_scalar(
                        qv, xsh(g, dh, dw, r_pool, pool_r), wap(g, t), None, MULT,
                    )
                    nc.gpsimd.tensor_tensor(accp, accp, qv, ADD)

            n1 = h1_r * 64
            if h1_r:
                acc1 = y2[:, r_h * 64 : r_h * 64 + n1]
                nc.vector.tensor_tensor(
                    acc1, slot[0][:, 0:n1], slot[1][:, 0:n1], ADD)
                for t in range(2, 9):
                    nc.vector.tensor_tensor(acc1, acc1, slot[t][:, 0:n1], ADD)
            if h2_r:
                acc2 = y2[:, r_h * 64 + n1 : (r_h + h_r) * 64]
                nc.gpsimd.tensor_tensor(
                    acc2, slot[0][:, n1:], slot[1][:, n1:], ADD)
                for t in range(2, 9):
                    nc.gpsimd.tensor_tensor(acc2, acc2, slot[t][:, n1:], ADD)

        def out_path(g):
            st = conv_state.pop(g)
            y2 = st["y2"]
            pe_r, dve_r, h1_r, h2_r, pool_r, e_r = _rows_for(g)
            bands = []
            if pe_r and not (PE_BAND_STREAM and g >= G - PE_STREAM_TAIL):
                bands.append((0, pe_r))
            if dve_r + h1_r:
                bands.append((pe_r, dve_r + h1_r))
            if h2_r + pool_r:
                bands.append((pe_r + dve_r + h1_r, h2_r + pool_r))
            for r0, nr in bands:
                nc.sync.dma_start(
                    ys[g][:, r0 * 64 : (r0 + nr) * 64],
                    y2[:, r0 * 64 : (r0 + nr) * 64],
                )

        def e_link(g, t):
            st = e_state[g]
            e_r = _rows_for(g)[5]
            r_e = 64 - e_r
            dst = ys[g][:, r_e * 64 : 4096]
            src = st["slots"][t]
            if t == 0:
                d = nc.gpsimd.dma_start(dst, src[:])
            else:
                d = nc.gpsimd.dma_start(dst, src[:], accum_op=ADD)
                add_dep_helper(d.ins, st["last"].ins, reason="accum-chain")
            st["last"] = d
            if t == 8:
                e_state.pop(g)

        flush = 8 if (E_ROWS or ROWS_SCHED) else 0
        for p in range(G + 1 + flush):
            if p < G:
                dma_issue(p)
                if DIAG_ON != "act" or p == 0:
                    diag_build(p, eng="dve" if p == 0 else None)
            if 1 <= p <= G:
                g = p - 1
                pe_conv(g)
                chains(g)
                out_path(g)
            if DIAG_ON == "act" and 0 < p < G:
                diag_build(p)
            for gl in sorted(e_state.keys()):
                t = p - 1 - gl
                if 0 <= t <= 8:
                    e_link(gl, t)

    return nc


# walrus setupSyncWait caps per engine struct: hoist excess waits onto
# injected same-engine Drains (Tile's epilogue Drain carries many waits,
# so Drain accepts them).
_WAIT_CAPS = {"PE": 1, "Activation": 1, "DVE": 1, "Pool": 1, "SP": 1}
_SPLIT_SEQ = [0]


def _split_waits(nc):
    fn = nc.m.functions[0]
    nsplit = 0
    for blk in fn.blocks:
        out = []
        changed = False
        for ins in blk.instructions:
            si = ins.sync_info
            waits = list(si.on_wait) if si is not None and si.on_wait else []
            eng = getattr(ins, "engine", None)
            engname = getattr(eng, "value", None) or str(eng)
            cap = _WAIT_CAPS.get(engname)
            if cap is not None and len(waits) > cap:
                excess, keep = waits[:-cap], waits[-cap:]
                for w in excess:
                    _SPLIT_SEQ[0] += 1
                    d = mybir.InstDrain(name=f"I-ws{_SPLIT_SEQ[0]}", ins=[], outs=[])
                    d.engine = eng
                    d.sync_info = mybir.SyncInfo(on_wait=[w], on_update=[])
                    out.append(d)
                ins.sync_info = mybir.SyncInfo(
                    on_wait=keep, on_update=list(si.on_update or [])
                )
                changed = True
                nsplit += 1
            out.append(ins)
        if changed:
            blk.instructions = out
    return nsplit


_NC_CACHE = None


def _get_nc():
    global _NC_CACHE
    if _NC_CACHE is None:
        nc = _build_nc()
        _split_waits(nc)
        _NC_CACHE = nc
    return _NC_CACHE


class Runner:
    """Persistent PJRT executor for an SPMD bass module (axon path)."""

    def __init__(self, nc, n_cores=8):
        import jax
        from jax.experimental.shard_map import shard_map
        from jax.sharding import Mesh, PartitionSpec
        from concourse import bass2jax

        bass2jax.install_neuronx_cc_hook()
        self.jax = jax
        self.nc = nc
        self.n = n_cores
        partition_name = (
            nc.partition_id_tensor.name if nc.partition_id_tensor else None
        )
        in_names, out_names, out_avals = [], [], []
        for alloc in nc.m.functions[0].allocations:
            if not isinstance(alloc, mybir.MemoryLocationSet):
                continue
            name = alloc.memorylocations[0].name
            if alloc.kind == "ExternalInput":
                if name != partition_name:
                    in_names.append(name)
            elif alloc.kind == "ExternalOutput":
                out_names.append(name)
                out_avals.append(
                    jax.core.ShapedArray(
                        tuple(alloc.tensor_shape), mybir.dt.np(alloc.dtype)
                    )
                )
        self.in_names = list(in_names)
        self.out_names = out_names
        self.out_avals = out_avals
        bind_in_names = list(in_names) + list(out_names)
        if partition_name is not None:
            bind_in_names.append(partition_name)
        bind_in_names = tuple(bind_in_names)
        n_params = len(in_names)
        n_outs = len(out_names)

        def _body(*args):
            operands = list(args)
            if partition_name is not None:
                operands.append(bass2jax.partition_id_tensor())
            outs = bass2jax._bass_exec_p.bind(
                *operands,
                out_avals=tuple(out_avals),
                in_names=bind_in_names,
                out_names=tuple(out_names),
                lowering_input_output_aliases=(),
                sim_require_finite=True,
                sim_require_nnan=True,
                nc=nc,
            )
            return tuple(outs)

        devices = jax.devices()[:n_cores]
        self.mesh = Mesh(np.asarray(devices), ("core",))
        self.spec = PartitionSpec("core")
        in_specs = (self.spec,) * (n_params + n_outs)
        out_specs = (self.spec,) * n_outs
        donate = tuple(range(n_params, n_params + n_outs))
        self.fn = jax.jit(
            shard_map(
                _body,
                mesh=self.mesh,
                in_specs=in_specs,
                out_specs=out_specs,
                check_rep=False,
            ),
            donate_argnums=donate,
            keep_unused=True,
        )
        sharding = jax.sharding.NamedSharding(self.mesh, self.spec)
        self.zeros_fn = jax.jit(
            lambda: tuple(
                self.jax.numpy.zeros((n_cores * a.shape[0], *a.shape[1:]), a.dtype)
                for a in out_avals
            ),
            out_shardings=(sharding,) * n_outs,
        )

    def put_inputs(self, in_maps):
        jax = self.jax
        sharding = jax.sharding.NamedSharding(self.mesh, self.spec)
        arrs = []
        for name in self.in_names:
            cat = np.concatenate([np.asarray(m[name]) for m in in_maps], axis=0)
            arrs.append(jax.device_put(cat, sharding))
        jax.block_until_ready(arrs)
        return arrs

    def __call__(self, dev_inputs):
        zs = self.zeros_fn()
        self.jax.block_until_ready(zs)
        out = self.fn(*dev_inputs, *zs)
        self.jax.block_until_ready(out)
        return out

    def time_it(self, dev_inputs, reps=10):
        import time as _t

        ts = []
        for _ in range(reps):
            zs = self.zeros_fn()
            self.jax.block_until_ready(zs)
            t0 = _t.perf_counter()
            out = self.fn(*dev_inputs, *zs)
            self.jax.block_until_ready(out)
            ts.append(_t.perf_counter() - t0)
        return ts

    def to_numpy(self, out):
        n = self.n
        return [
            {
                name: np.asarray(out[i]).reshape(n, *self.out_avals[i].shape)[c]
                for i, name in enumerate(self.out_names)
            }
            for c in range(n)
        ]


_RUNNER = None


def _get_runner():
    global _RUNNER
    if _RUNNER is None:
        _RUNNER = Runner(_get_nc(), B)
    return _RUNNER


def _prep_wb(w, b):
    # ws[p, g*9 + t] = w[2g + p//64, t//3, t%3, p%64]
    w = np.asarray(w, dtype=np.float32).reshape(G, 2, 9, C)
    ws = np.ascontiguousarray(w.transpose(1, 3, 0, 2).reshape(128, G * 9))
    b = np.asarray(b, dtype=np.float32).reshape(G, 2, C)
    bs = np.ascontiguousarray(b.transpose(1, 2, 0).reshape(128, G))
    return ws, bs


def _prep_x(xi):
    """[H,W,D,C] f32 -> [G, 128, SLAB] fp16 padded slab."""
    # (h, w, d, c) -> (g, dp, c, h, w)
    xt = xi.transpose(2, 3, 0, 1).reshape(G, 2, C, H, W)
    slab = np.zeros((G, 2, C, SLAB), dtype=np.float16)
    sv = slab[:, :, :, DATA0 : DATA0 + 64 * RS].reshape(G, 2, C, 64, RS)
    sv[:, :, :, :, 0:64] = xt.astype(np.float16)
    return slab.reshape(G, 128, SLAB)


def _post_y(ysg):
    """[G, 128, 4096] fp16 -> [H,W,D,C] f32."""
    y = ysg.reshape(G, 2, C, H, W).astype(np.float32)
    # (g, dp, c, h, w) -> (h, w, g, dp, c) -> [H, W, D, C]
    return np.ascontiguousarray(y.transpose(3, 4, 0, 1, 2).reshape(H, W, D, C))


def _in_maps(inputs):
    x = np.asarray(inputs["x"], dtype=np.float32)
    ws, bs = _prep_wb(inputs["w"], inputs["b"])
    wbs = np.ascontiguousarray(np.concatenate([ws, bs], axis=1))
    return [{"xs": _prep_x(x[i]), "wbs": wbs} for i in range(B)]


def kernel(**inputs) -> np.ndarray:
    r = _get_runner()
    dev = r.put_inputs(_in_maps(inputs))
    res = r.to_numpy(r(dev))
    return np.stack([_post_y(m["ys"]) for m in res], axis=0)
